# revision 14
# baseline (speedup 1.0000x reference)
"""Distributed single-head causal attention for Trainium2 (8 NeuronCores).

Problem: x:[4,2048,1024] f32, Wq/Wk/Wv/Wo:[1024,1024], b*:[1024]
  q = x@Wq.T+bq; k = x@Wk.T+bk; v = x@Wv.T+bv
  scores = (q@k.T)/sqrt(1024) causal-masked; out = softmax(scores)@v @Wo.T + bo

Sharding (data-parallel pairs, block-parity compact, strict-SPMD):
  8 cores = 4 batches x 2 cores/batch. The 16 query blocks (128 rows) of a
  batch split by parity: even core takes even blocks, odd core odd blocks.
  Every core runs 8 "slots" with the compile-time schedule: slot s handles
  query block 2s+parity and attends E-compact key blocks [0..s] plus
  O-compact key blocks [0..s] (E/O = even/odd logical 128-blocks of keys,
  each stored compacted). Causality lives entirely in the host-built mask:
  the diagonal block is in the core's own parity array; the opposite
  parity's boundary block s is fully open (odd cores) or fully masked
  (even cores). Identical instruction streams, balanced work.

K is projected only for the core's own parity blocks (x-parity input is
shared with Q proj) and exchanged within the pair by a 2MB DRAM-bounce
AllGather; rank order makes ccout[0]=K-even, ccout[1]=K-odd on both cores,
so the readback is rank-free. V is projected fully on-core from an
E|O-compact x copy while the collective is in flight (~66us measured
wall for a pair AllGather: 26us start + ~100GB/s), so the exchange hides
behind V+Q projection. No max-subtraction in softmax (scores ~N(0,1),
exp overflow-safe); unnormalized exp rows are transposed via xbar DMA and
normalization (1/l) folds into the output projection.

Per-core PE work: K-own 65536cyc + V-full 131072 + Q 65536 + scores 73728
+ attnV 73728 + out 65536 = 475k cycles (~198us @2.4GHz).
"""

import sys

if "/opt/trn_rl_repo" not in sys.path:
    sys.path.insert(0, "/opt/trn_rl_repo")

import numpy as np
import ml_dtypes

import concourse.bass as bass
import concourse.mybir as mybir
from concourse import bacc
from concourse.bass_utils import run_bass_kernel_spmd
from concourse.tile import TileContext

B, S, D = 4, 2048, 1024
NB = S // 128
NSLOT = 8
EC = D // 128
F32 = mybir.dt.float32
BF16 = mybir.dt.bfloat16
NEG = -1.0e9
GROUPS = [[0, 1], [2, 3], [4, 5], [6, 7]]

_compiled = None


def _build():
    nc = bacc.Bacc("TRN2", target_bir_lowering=False, debug=False, num_devices=8)

    # xpT: parity-compact x (the core's own 8 blocks) - feeds Q proj AND K-own
    xpT = nc.dram_tensor("xpT", [128, EC, 1024], BF16, kind="ExternalInput")
    # xeoT: full x, E-compact | O-compact arrangement - feeds V proj
    xeoT = nc.dram_tensor("xeoT", [128, EC, S], BF16, kind="ExternalInput")
    wqT = nc.dram_tensor("wqT", [128, EC, D], BF16, kind="ExternalInput")
    wkT = nc.dram_tensor("wkT", [128, EC, D], BF16, kind="ExternalInput")
    wvT = nc.dram_tensor("wvT", [128, EC, D], BF16, kind="ExternalInput")
    woT = nc.dram_tensor("woT", [128, EC, D], BF16, kind="ExternalInput")
    bq_d = nc.dram_tensor("bq", [128, EC], F32, kind="ExternalInput")
    bk_d = nc.dram_tensor("bk", [128, EC], F32, kind="ExternalInput")
    bv_d = nc.dram_tensor("bv", [1, D], F32, kind="ExternalInput")
    bo_d = nc.dram_tensor("bo", [1, D], F32, kind="ExternalInput")
    # mask[:, s, 0:128] masks E-compact block s; [:, s, 128:256] O-compact s
    mask_d = nc.dram_tensor("mask", [128, NSLOT, 256], F32, kind="ExternalInput")
    out_d = nc.dram_tensor("out", [NSLOT * 128, D], F32, kind="ExternalOutput")

    inv = 1.0 / 32.0

    with TileContext(nc) as tc:
        with (
            tc.tile_pool(name="persist", bufs=1) as persist,
            tc.tile_pool(name="small", bufs=1) as small,
            tc.tile_pool(name="dram", bufs=1, space="DRAM") as dram,
        ):
            QT = persist.tile([128, EC, 1024], BF16, tag="QT")
            KTE = persist.tile([128, EC, 1024], BF16, tag="KTE")
            KTO = persist.tile([128, EC, 1024], BF16, tag="KTO")
            # V blocks 0..7 = E-compact, 8..15 = O-compact (from xeoT order)
            V = persist.tile([128, NB, D], BF16, tag="V")
            MASK = small.tile([128, NSLOT, 256], F32, tag="MASK")
            BQ = small.tile([128, EC], F32, tag="BQ")
            BK = small.tile([128, EC], F32, tag="BK")
            RL = small.tile([128, NSLOT], F32, tag="RL")
            BOF = small.tile([128, D], F32, tag="BOF")

            bounce_in = dram.tile([128, EC, 1024], BF16, name="bnc_in")
            bounce_out = dram.tile([2, 128, EC, 1024], BF16, name="bnc_out")

            # ---- phase A: K-own proj -> bounce -> pair AllGather;
            #      V-full + Q proj overlap the collective ----
            with (
                tc.tile_pool(name="xin", bufs=1) as xin,
                tc.tile_pool(name="wts", bufs=1) as wts,
                tc.tile_pool(name="ko", bufs=1) as ko_pool,
                tc.tile_pool(name="pa_psum", bufs=8, space="PSUM") as pa_psum,
            ):
                XP = xin.tile([128, EC, 1024], BF16, tag="XP")
                WQ = wts.tile([128, EC, D], BF16, tag="WQ")
                WK = wts.tile([128, EC, D], BF16, tag="WK")
                WV = wts.tile([128, EC, D], BF16, tag="WV")
                BVF = xin.tile([128, D], F32, tag="BVF")
                KTOWN = ko_pool.tile([128, EC, 1024], BF16, tag="KTOWN")

                bv_row = small.tile([1, D], F32, tag="bv_row")
                nc.sync.dma_start(out=bv_row[:, :], in_=bv_d[:, :])
                nc.gpsimd.partition_broadcast(BVF[:, :], bv_row[:1, :])
                nc.sync.dma_start(out=BK[:, :], in_=bk_d[:, :])
                # xp + WK feed the K-own waves first; strip-0 only needs the
                # low column halves of XP, so load those first (sync) and the
                # high halves on the scalar queue; WK/WV chunks split across
                # both queues so the first waves run at combined bandwidth
                for dc in range(EC):
                    nc.sync.dma_start(out=XP[:, dc, 0:512], in_=xpT[:, dc, 0:512])
                for dc in range(0, EC, 2):
                    nc.scalar.dma_start(out=WK[:, dc, :], in_=wkT[:, dc, :])
                for dc in range(EC):
                    nc.scalar.dma_start(
                        out=XP[:, dc, 512:1024], in_=xpT[:, dc, 512:1024]
                    )
                for dc in range(1, EC, 2):
                    nc.sync.dma_start(out=WK[:, dc, :], in_=wkT[:, dc, :])
                for dc in range(EC):
                    eng = nc.scalar if dc % 2 == 1 else nc.sync
                    eng.dma_start(out=WV[:, dc, :], in_=wvT[:, dc, :])

                # K-own: 2 x 512-col strips of the parity-compact x
                for th in range(2):
                    for wv2 in range(2):
                        ec0 = 4 * wv2
                        pss = [
                            pa_psum.tile(
                                [128, 512], F32, tag="pa", name=f"pak{th}_{wv2}_{i}"
                            )
                            for i in range(4)
                        ]
                        for dc in range(EC):
                            for i in range(4):
                                nc.tensor.matmul(
                                    pss[i][:, :],
                                    WK[:, dc, (ec0 + i) * 128 : (ec0 + i + 1) * 128],
                                    XP[:, dc, th * 512 : (th + 1) * 512],
                                    start=(dc == 0),
                                    stop=(dc == EC - 1),
                                )
                        for i in range(4):
                            ec = ec0 + i
                            nc.vector.tensor_scalar(
                                out=KTOWN[:, ec, th * 512 : (th + 1) * 512],
                                in0=pss[i][:, :],
                                scalar1=BK[:, ec : ec + 1],
                                scalar2=None,
                                op0=mybir.AluOpType.add,
                            )
                    # ship this strip to the bounce buffer as soon as done
                    nc.scalar.dma_start(
                        out=bounce_in[:, :, th * 512 : (th + 1) * 512],
                        in_=KTOWN[:, :, th * 512 : (th + 1) * 512],
                    )

                # pair exchange: ccout[0] = K-even, ccout[1] = K-odd on BOTH
                # cores (AllGather output is rank-ordered) -> rank-free readback
                nc.gpsimd.collective_compute(
                    "AllGather",
                    mybir.AluOpType.bypass,
                    replica_groups=GROUPS,
                    ins=[bounce_in.opt()],
                    outs=[bounce_out.opt()],
                )

                for dc in range(EC):
                    nc.scalar.dma_start(out=WQ[:, dc, :], in_=wqT[:, dc, :])

                # V-full from the E|O-compact x copy, streamed in strips
                with tc.tile_pool(name="xeo", bufs=2) as xeo_pool:
                    for th in range(4):
                        if th == 2:
                            bq_raw = small.tile([128, EC], F32, tag="bq_raw")
                            nc.sync.dma_start(out=bq_raw[:, :], in_=bq_d[:, :])
                            nc.scalar.mul(BQ[:, :], bq_raw[:, :], inv)
                            nc.sync.dma_start(out=MASK[:, :, :], in_=mask_d[:, :, :])
                            bo_row = small.tile([1, D], F32, tag="bo_row")
                            nc.sync.dma_start(out=bo_row[:, :], in_=bo_d[:, :])
                            nc.gpsimd.partition_broadcast(BOF[:, :], bo_row[:1, :])
                        XEs = xeo_pool.tile(
                            [128, EC, 512], BF16, tag="xeo", name=f"xeo{th}"
                        )
                        for dc in range(EC):
                            nc.sync.dma_start(
                                out=XEs[:, dc, :],
                                in_=xeoT[:, dc, th * 512 : (th + 1) * 512],
                            )
                        for wv2 in range(2):
                            tb0 = 4 * th + 2 * wv2
                            pss = [
                                pa_psum.tile(
                                    [128, 512], F32, tag="pa",
                                    name=f"pavf{th}_{wv2}_{i}",
                                )
                                for i in range(4)
                            ]
                            for dc in range(EC):
                                for i, (tb, dh) in enumerate(
                                    [(tb0, 0), (tb0, 1), (tb0 + 1, 0), (tb0 + 1, 1)]
                                ):
                                    nc.tensor.matmul(
                                        pss[i][:, :],
                                        XEs[:, dc, (tb - 4 * th) * 128 : (tb - 4 * th + 1) * 128],
                                        WV[:, dc, dh * 512 : (dh + 1) * 512],
                                        start=(dc == 0),
                                        stop=(dc == EC - 1),
                                    )
                            for i, (tb, dh) in enumerate(
                                [(tb0, 0), (tb0, 1), (tb0 + 1, 0), (tb0 + 1, 1)]
                            ):
                                nc.vector.tensor_tensor(
                                    out=V[:, tb, dh * 512 : (dh + 1) * 512],
                                    in0=pss[i][:, :],
                                    in1=BVF[:, dh * 512 : (dh + 1) * 512],
                                    op=mybir.AluOpType.add,
                                )

                # QT (x 1/32, +bq/32) from the same parity-compact x
                for sh in range(2):
                    for w in range(2):
                        ec0 = 4 * w
                        pss = [
                            pa_psum.tile(
                                [128, 512], F32, tag="pa", name=f"paq{sh}_{w}_{i}"
                            )
                            for i in range(4)
                        ]
                        for dc in range(EC):
                            for i in range(4):
                                nc.tensor.matmul(
                                    pss[i][:, :],
                                    WQ[:, dc, (ec0 + i) * 128 : (ec0 + i + 1) * 128],
                                    XP[:, dc, sh * 512 : (sh + 1) * 512],
                                    start=(dc == 0),
                                    stop=(dc == EC - 1),
                                )
                        for i in range(4):
                            ec = ec0 + i
                            nc.vector.tensor_scalar(
                                out=QT[:, ec, sh * 512 : (sh + 1) * 512],
                                in0=pss[i][:, :],
                                scalar1=inv,
                                scalar2=BQ[:, ec : ec + 1],
                                op0=mybir.AluOpType.mult,
                                op1=mybir.AluOpType.add,
                            )

                # readback both parity arrays, block by block ascending so
                # early (narrow) slots unblock first; E on sync, O on scalar
                for b in range(8):
                    nc.sync.dma_start(
                        out=KTE[:, :, b * 128 : (b + 1) * 128],
                        in_=bounce_out[0, :, :, b * 128 : (b + 1) * 128],
                    )
                    nc.scalar.dma_start(
                        out=KTO[:, :, b * 128 : (b + 1) * 128],
                        in_=bounce_out[1, :, :, b * 128 : (b + 1) * 128],
                    )

            # ---- phase B + C: attention + output projection ----
            with (
                tc.tile_pool(name="wo", bufs=1) as wo_pool,
                tc.tile_pool(name="att", bufs=5) as att_pool,
                tc.tile_pool(name="attT", bufs=2) as attT_pool,
                tc.tile_pool(name="ctx", bufs=1) as ctx_pool,
                tc.tile_pool(name="stat", bufs=1) as stat_pool,
                tc.tile_pool(name="sc_psum", bufs=3, space="PSUM") as sc_psum,
                tc.tile_pool(name="mm_psum", bufs=2, space="PSUM") as mm_psum,
                tc.tile_pool(name="outbuf", bufs=2) as out_pool,
            ):
                WO = wo_pool.tile([128, EC, D], BF16, tag="WO")
                for dc in range(EC):
                    nc.sync.dma_start(out=WO[:, dc, :], in_=woT[:, dc, :])
                CTXT = ctx_pool.tile([128, EC, 1024], BF16, tag="CTXT")
                LSUM = stat_pool.tile([128, 2 * NSLOT], F32, tag="LS")
                LTOT = stat_pool.tile([128, NSLOT], F32, tag="LT")

                def out_proj(slot):
                    OUTS = out_pool.tile([128, D], F32, tag="outs")
                    for eh in range(2):
                        ps = mm_psum.tile([128, 512], F32, tag="mm")
                        for dc in range(EC):
                            nc.tensor.matmul(
                                ps[:, :],
                                CTXT[:, dc, slot * 128 : (slot + 1) * 128],
                                WO[:, dc, eh * 512 : (eh + 1) * 512],
                                start=(dc == 0),
                                stop=(dc == EC - 1),
                            )
                        nc.vector.scalar_tensor_tensor(
                            out=OUTS[:, eh * 512 : (eh + 1) * 512],
                            in0=ps[:, :],
                            scalar=RL[:, slot : slot + 1],
                            in1=BOF[:, eh * 512 : (eh + 1) * 512],
                            op0=mybir.AluOpType.mult,
                            op1=mybir.AluOpType.add,
                        )
                        nc.scalar.dma_start(
                            out=out_d[
                                slot * 128 : (slot + 1) * 128,
                                eh * 512 : (eh + 1) * 512,
                            ],
                            in_=OUTS[:, eh * 512 : (eh + 1) * 512],
                        )

                for g in range(2):
                    # ATT_T rows 0..7: E-compact blocks, rows 8..15: O-compact
                    ATT_T = attT_pool.tile([128, NB, 512], BF16, tag="attT")
                    # widest slot first: its long tail overlaps narrower slots
                    for j in (3, 2, 1, 0):
                        slot = g * 4 + j
                        W = 128 * (slot + 1)  # per-parity width
                        ATT = att_pool.tile([128, S], BF16, tag="att")

                        scE = sc_psum.tile([128, 1024], F32, tag="sc")
                        scO = sc_psum.tile([128, 1024], F32, tag="sc")
                        # ec-outer; E+O chunks share the per-ec LDWEIGHTS
                        for ec in range(EC):
                            for c0 in range(0, W, 512):
                                cw = min(512, W - c0)
                                nc.tensor.matmul(
                                    scE[:, c0 : c0 + cw],
                                    QT[:, ec, slot * 128 : (slot + 1) * 128],
                                    KTE[:, ec, c0 : c0 + cw],
                                    start=(ec == 0),
                                    stop=(ec == EC - 1),
                                )
                            for c0 in range(0, W, 512):
                                cw = min(512, W - c0)
                                nc.tensor.matmul(
                                    scO[:, c0 : c0 + cw],
                                    QT[:, ec, slot * 128 : (slot + 1) * 128],
                                    KTO[:, ec, c0 : c0 + cw],
                                    start=(ec == 0),
                                    stop=(ec == EC - 1),
                                )
                        nc.vector.tensor_tensor(
                            out=scE[:, W - 128 : W],
                            in0=scE[:, W - 128 : W],
                            in1=MASK[:, slot, 0:128],
                            op=mybir.AluOpType.add,
                        )
                        nc.vector.tensor_tensor(
                            out=scO[:, W - 128 : W],
                            in0=scO[:, W - 128 : W],
                            in1=MASK[:, slot, 128:256],
                            op=mybir.AluOpType.add,
                        )
                        # no max-subtraction: scores ~ N(0,1), exp is safe;
                        # each parity's exp+transpose fires independently
                        nc.scalar.activation(
                            ATT[:, 0:W],
                            scE[:, :W],
                            mybir.ActivationFunctionType.Exp,
                            bias=0.0,
                            scale=1.0,
                            accum_out=LSUM[:, 2 * slot : 2 * slot + 1],
                        )
                        nc.sync.dma_start_transpose(
                            ATT_T[:, 0 : W // 128, j * 128 : (j + 1) * 128],
                            ATT[:, 0:W],
                        )
                        nc.scalar.activation(
                            ATT[:, 1024 : 1024 + W],
                            scO[:, :W],
                            mybir.ActivationFunctionType.Exp,
                            bias=0.0,
                            scale=1.0,
                            accum_out=LSUM[:, 2 * slot + 1 : 2 * slot + 2],
                        )
                        nc.sync.dma_start_transpose(
                            ATT_T[:, 8 : 8 + W // 128, j * 128 : (j + 1) * 128],
                            ATT[:, 1024 : 1024 + W],
                        )
                        nc.vector.tensor_tensor(
                            out=LTOT[:, slot : slot + 1],
                            in0=LSUM[:, 2 * slot : 2 * slot + 1],
                            in1=LSUM[:, 2 * slot + 1 : 2 * slot + 2],
                            op=mybir.AluOpType.add,
                        )
                        nc.vector.reciprocal(
                            RL[:, slot : slot + 1], LTOT[:, slot : slot + 1]
                        )

                    # attnV: E pass then O pass, one continued accumulation;
                    # compact ragged: slot j covers parity blocks [0..g*4+j]
                    ntc = g * 4 + 4
                    for dc in range(EC):
                        ps = mm_psum.tile([128, 512], F32, tag="mm")
                        for pi, base in ((0, 0), (1, 8)):
                            for tcn in range(ntc):
                                scol = max(0, tcn - g * 4) * 128
                                nc.tensor.matmul(
                                    ps[:, scol:512],
                                    V[:, base + tcn, dc * 128 : (dc + 1) * 128],
                                    ATT_T[:, base + tcn, scol:512],
                                    start=(pi == 0 and tcn == 0),
                                    stop=(pi == 1 and tcn == ntc - 1),
                                )
                        nc.vector.tensor_copy(
                            CTXT[:, dc, g * 512 : (g + 1) * 512], ps[:, :]
                        )
                    for j in range(4):
                        out_proj(g * 4 + j)

    nc.compile()
    return nc


def _core_blocks(core):
    parity = core % 2  # even core (pair rank 0) -> even blocks
    return [2 * s + parity for s in range(NSLOT)]


def _make_in_maps(x, Wq, bq, Wk, bk, Wv, bv, Wo, bo):
    bf = ml_dtypes.bfloat16

    def wt_layout(W):
        return np.ascontiguousarray(
            W.T.astype(bf).reshape(EC, 128, D).transpose(1, 0, 2)
        )

    def xT_layout(xrows, n):
        return np.ascontiguousarray(
            xrows.T.astype(bf).reshape(EC, 128, n).transpose(1, 0, 2)
        )

    wq_l, wk_l, wv_l, wo_l = (wt_layout(W) for W in (Wq, Wk, Wv, Wo))
    bq_l = np.ascontiguousarray(bq.reshape(EC, 128).T.astype(np.float32))
    bk_l = np.ascontiguousarray(bk.reshape(EC, 128).T.astype(np.float32))
    bv_l = np.ascontiguousarray(bv.reshape(1, D).astype(np.float32))
    bo_l = np.ascontiguousarray(bo.reshape(1, D).astype(np.float32))

    in_maps = []
    for core in range(8):
        b = core // 2
        p = core % 2
        blocks = _core_blocks(core)
        xb = np.asarray(x[b], dtype=np.float32)
        # parity-compact x: own blocks (used for Q and K-own)
        xp = np.concatenate([xb[bl * 128 : (bl + 1) * 128] for bl in blocks], axis=0)
        # E-compact | O-compact full x (used for V)
        xeo = np.concatenate(
            [xb[bl * 128 : (bl + 1) * 128] for bl in range(0, NB, 2)]
            + [xb[bl * 128 : (bl + 1) * 128] for bl in range(1, NB, 2)],
            axis=0,
        )
        # mask: [:, s, 0:128] on E-compact block s, [:, s, 128:256] on O s.
        # diagonal lives in the own-parity array; the other parity's block s
        # is fully open for odd cores, fully masked for even cores.
        mask = np.zeros((128, NSLOT, 256), np.float32)
        r = np.arange(128)[:, None]
        o = np.arange(128)[None, :]
        diag = np.where(o <= r, 0.0, NEG)
        for s_i in range(NSLOT):
            if p == 0:
                mask[:, s_i, 0:128] = diag
                mask[:, s_i, 128:256] = NEG
            else:
                mask[:, s_i, 0:128] = 0.0
                mask[:, s_i, 128:256] = diag
        in_maps.append(
            {
                "xpT": xT_layout(xp, 1024),
                "xeoT": xT_layout(xeo, S),
                "wqT": wq_l,
                "wkT": wk_l,
                "wvT": wv_l,
                "woT": wo_l,
                "bq": bq_l,
                "bk": bk_l,
                "bv": bv_l,
                "bo": bo_l,
                "mask": mask,
            }
        )
    return in_maps


def _run(inputs, trace=False):
    global _compiled
    if _compiled is None:
        _compiled = _build()
    nc = _compiled
    in_maps = _make_in_maps(**inputs)
    res = run_bass_kernel_spmd(nc, in_maps, core_ids=list(range(8)), trace=trace)
    out = np.zeros((B, S, D), np.float32)
    for core in range(8):
        b = core // 2
        o = res.results[core]["out"]
        for s_i, bl in enumerate(_core_blocks(core)):
            out[b, bl * 128 : (bl + 1) * 128, :] = o[s_i * 128 : (s_i + 1) * 128, :]
    return out, res


def kernel(**inputs):
    out, _ = _run(inputs, trace=False)
    return out


# revision 15
# speedup vs baseline: 1.0062x; 1.0062x over previous
"""Distributed single-head causal attention for Trainium2 (8 NeuronCores).

Problem: x:[4,2048,1024] f32, Wq/Wk/Wv/Wo:[1024,1024], b*:[1024]
  q = x@Wq.T+bq; k = x@Wk.T+bk; v = x@Wv.T+bv
  scores = (q@k.T)/sqrt(1024) causal-masked; out = softmax(scores)@v @Wo.T + bo

Sharding (data-parallel pairs, block-parity compact, strict-SPMD):
  8 cores = 4 batches x 2 cores/batch. The 16 query blocks (128 rows) of a
  batch split by parity: even core takes even blocks, odd core odd blocks.
  Every core runs 8 "slots" with the compile-time schedule: slot s handles
  query block 2s+parity and attends E-compact key blocks [0..s] plus
  O-compact key blocks [0..s] (E/O = even/odd logical 128-blocks of keys,
  each stored compacted). Causality lives entirely in the host-built mask:
  the diagonal block is in the core's own parity array; the opposite
  parity's boundary block s is fully open (odd cores) or fully masked
  (even cores). Identical instruction streams, balanced work.

K is projected only for the core's own parity blocks (x-parity input is
shared with Q proj) and exchanged within the pair by a 2MB DRAM-bounce
AllGather; rank order makes ccout[0]=K-even, ccout[1]=K-odd on both cores,
so the readback is rank-free. V is projected fully on-core from an
E|O-compact x copy while the collective is in flight (~66us measured
wall for a pair AllGather: 26us start + ~100GB/s), so the exchange hides
behind V+Q projection. No max-subtraction in softmax (scores ~N(0,1),
exp overflow-safe); unnormalized exp rows are transposed via xbar DMA and
normalization (1/l) folds into the output projection.

Per-core PE work: K-own 65536cyc + V-full 131072 + Q 65536 + scores 73728
+ attnV 73728 + out 65536 = 475k cycles (~198us @2.4GHz).
"""

import sys

if "/opt/trn_rl_repo" not in sys.path:
    sys.path.insert(0, "/opt/trn_rl_repo")

import numpy as np
import ml_dtypes

import concourse.bass as bass
import concourse.mybir as mybir
from concourse import bacc
from concourse.bass_utils import run_bass_kernel_spmd
from concourse.tile import TileContext

B, S, D = 4, 2048, 1024
NB = S // 128
NSLOT = 8
EC = D // 128
F32 = mybir.dt.float32
BF16 = mybir.dt.bfloat16
NEG = -1.0e9
GROUPS = [[0, 1], [2, 3], [4, 5], [6, 7]]

_compiled = None


def _build():
    nc = bacc.Bacc("TRN2", target_bir_lowering=False, debug=False, num_devices=8)

    # xpT: parity-compact x (the core's own 8 blocks) - feeds Q proj AND K-own
    xpT = nc.dram_tensor("xpT", [128, EC, 1024], BF16, kind="ExternalInput")
    # xeoT: full x, E-compact | O-compact arrangement - feeds V proj
    xeoT = nc.dram_tensor("xeoT", [128, EC, S], BF16, kind="ExternalInput")
    wqT = nc.dram_tensor("wqT", [128, EC, D], BF16, kind="ExternalInput")
    wkT = nc.dram_tensor("wkT", [128, EC, D], BF16, kind="ExternalInput")
    wvT = nc.dram_tensor("wvT", [128, EC, D], BF16, kind="ExternalInput")
    woT = nc.dram_tensor("woT", [128, EC, D], BF16, kind="ExternalInput")
    bq_d = nc.dram_tensor("bq", [128, EC], F32, kind="ExternalInput")
    bk_d = nc.dram_tensor("bk", [128, EC], F32, kind="ExternalInput")
    bv_d = nc.dram_tensor("bv", [1, D], F32, kind="ExternalInput")
    bo_d = nc.dram_tensor("bo", [1, D], F32, kind="ExternalInput")
    # mask[:, s, 0:128] masks E-compact block s; [:, s, 128:256] O-compact s
    mask_d = nc.dram_tensor("mask", [128, NSLOT, 256], F32, kind="ExternalInput")
    out_d = nc.dram_tensor("out", [NSLOT * 128, D], F32, kind="ExternalOutput")

    inv = 1.0 / 32.0

    with TileContext(nc) as tc:
        with (
            tc.tile_pool(name="persist", bufs=1) as persist,
            tc.tile_pool(name="small", bufs=1) as small,
            tc.tile_pool(name="dram", bufs=1, space="DRAM") as dram,
        ):
            QT = persist.tile([128, EC, 1024], BF16, tag="QT")
            KTE = persist.tile([128, EC, 1024], BF16, tag="KTE")
            KTO = persist.tile([128, EC, 1024], BF16, tag="KTO")
            # V blocks 0..7 = E-compact, 8..15 = O-compact (from xeoT order)
            V = persist.tile([128, NB, D], BF16, tag="V")
            MASK = small.tile([128, NSLOT, 256], F32, tag="MASK")
            BQ = small.tile([128, EC], F32, tag="BQ")
            BK = small.tile([128, EC], F32, tag="BK")
            RL = small.tile([128, NSLOT], F32, tag="RL")
            BOF = small.tile([128, D], F32, tag="BOF")

            bounce_in = dram.tile([128, EC, 1024], BF16, name="bnc_in")
            bounce_out = dram.tile([2, 128, EC, 1024], BF16, name="bnc_out")

            # ---- phase A: K-own proj -> bounce -> pair AllGather;
            #      V-full + Q proj overlap the collective ----
            with (
                tc.tile_pool(name="xin", bufs=1) as xin,
                tc.tile_pool(name="wts", bufs=1) as wts,
                tc.tile_pool(name="ko", bufs=1) as ko_pool,
                tc.tile_pool(name="pa_psum", bufs=8, space="PSUM") as pa_psum,
            ):
                XP = xin.tile([128, EC, 1024], BF16, tag="XP")
                WQ = wts.tile([128, EC, D], BF16, tag="WQ")
                WK = wts.tile([128, EC, D], BF16, tag="WK")
                WV = wts.tile([128, EC, D], BF16, tag="WV")
                BVF = xin.tile([128, D], F32, tag="BVF")
                KTOWN = ko_pool.tile([128, EC, 1024], BF16, tag="KTOWN")

                bv_row = small.tile([1, D], F32, tag="bv_row")
                nc.sync.dma_start(out=bv_row[:, :], in_=bv_d[:, :])
                nc.gpsimd.partition_broadcast(BVF[:, :], bv_row[:1, :])
                nc.sync.dma_start(out=BK[:, :], in_=bk_d[:, :])
                # xp + WK feed the K-own waves first; chunks split across
                # both queues so the first waves run at combined bandwidth
                for dc in range(EC):
                    nc.sync.dma_start(out=XP[:, dc, :], in_=xpT[:, dc, :])
                for dc in range(EC):
                    eng = nc.scalar if dc % 2 == 0 else nc.sync
                    eng.dma_start(out=WK[:, dc, :], in_=wkT[:, dc, :])
                for dc in range(EC):
                    eng = nc.scalar if dc % 2 == 1 else nc.sync
                    eng.dma_start(out=WV[:, dc, :], in_=wvT[:, dc, :])

                # K-own: 2 x 512-col strips of the parity-compact x
                for th in range(2):
                    for wv2 in range(2):
                        ec0 = 4 * wv2
                        pss = [
                            pa_psum.tile(
                                [128, 512], F32, tag="pa", name=f"pak{th}_{wv2}_{i}"
                            )
                            for i in range(4)
                        ]
                        for dc in range(EC):
                            for i in range(4):
                                nc.tensor.matmul(
                                    pss[i][:, :],
                                    WK[:, dc, (ec0 + i) * 128 : (ec0 + i + 1) * 128],
                                    XP[:, dc, th * 512 : (th + 1) * 512],
                                    start=(dc == 0),
                                    stop=(dc == EC - 1),
                                )
                        for i in range(4):
                            ec = ec0 + i
                            nc.vector.tensor_scalar(
                                out=KTOWN[:, ec, th * 512 : (th + 1) * 512],
                                in0=pss[i][:, :],
                                scalar1=BK[:, ec : ec + 1],
                                scalar2=None,
                                op0=mybir.AluOpType.add,
                            )
                    # ship this strip to the bounce buffer as soon as done
                    nc.scalar.dma_start(
                        out=bounce_in[:, :, th * 512 : (th + 1) * 512],
                        in_=KTOWN[:, :, th * 512 : (th + 1) * 512],
                    )

                # pair exchange: ccout[0] = K-even, ccout[1] = K-odd on BOTH
                # cores (AllGather output is rank-ordered) -> rank-free readback
                nc.gpsimd.collective_compute(
                    "AllGather",
                    mybir.AluOpType.bypass,
                    replica_groups=GROUPS,
                    ins=[bounce_in.opt()],
                    outs=[bounce_out.opt()],
                )

                for dc in range(EC):
                    nc.scalar.dma_start(out=WQ[:, dc, :], in_=wqT[:, dc, :])

                # V-full from the E|O-compact x copy, streamed in strips
                with tc.tile_pool(name="xeo", bufs=2) as xeo_pool:
                    for th in range(4):
                        if th == 2:
                            bq_raw = small.tile([128, EC], F32, tag="bq_raw")
                            nc.sync.dma_start(out=bq_raw[:, :], in_=bq_d[:, :])
                            nc.scalar.mul(BQ[:, :], bq_raw[:, :], inv)
                            nc.sync.dma_start(out=MASK[:, :, :], in_=mask_d[:, :, :])
                            bo_row = small.tile([1, D], F32, tag="bo_row")
                            nc.sync.dma_start(out=bo_row[:, :], in_=bo_d[:, :])
                            nc.gpsimd.partition_broadcast(BOF[:, :], bo_row[:1, :])
                        XEs = xeo_pool.tile(
                            [128, EC, 512], BF16, tag="xeo", name=f"xeo{th}"
                        )
                        for dc in range(EC):
                            nc.sync.dma_start(
                                out=XEs[:, dc, :],
                                in_=xeoT[:, dc, th * 512 : (th + 1) * 512],
                            )
                        for wv2 in range(2):
                            tb0 = 4 * th + 2 * wv2
                            pss = [
                                pa_psum.tile(
                                    [128, 512], F32, tag="pa",
                                    name=f"pavf{th}_{wv2}_{i}",
                                )
                                for i in range(4)
                            ]
                            for dc in range(EC):
                                for i, (tb, dh) in enumerate(
                                    [(tb0, 0), (tb0, 1), (tb0 + 1, 0), (tb0 + 1, 1)]
                                ):
                                    nc.tensor.matmul(
                                        pss[i][:, :],
                                        XEs[:, dc, (tb - 4 * th) * 128 : (tb - 4 * th + 1) * 128],
                                        WV[:, dc, dh * 512 : (dh + 1) * 512],
                                        start=(dc == 0),
                                        stop=(dc == EC - 1),
                                    )
                            for i, (tb, dh) in enumerate(
                                [(tb0, 0), (tb0, 1), (tb0 + 1, 0), (tb0 + 1, 1)]
                            ):
                                nc.vector.tensor_tensor(
                                    out=V[:, tb, dh * 512 : (dh + 1) * 512],
                                    in0=pss[i][:, :],
                                    in1=BVF[:, dh * 512 : (dh + 1) * 512],
                                    op=mybir.AluOpType.add,
                                )

                # QT (x 1/32, +bq/32) from the same parity-compact x
                for sh in range(2):
                    for w in range(2):
                        ec0 = 4 * w
                        pss = [
                            pa_psum.tile(
                                [128, 512], F32, tag="pa", name=f"paq{sh}_{w}_{i}"
                            )
                            for i in range(4)
                        ]
                        for dc in range(EC):
                            for i in range(4):
                                nc.tensor.matmul(
                                    pss[i][:, :],
                                    WQ[:, dc, (ec0 + i) * 128 : (ec0 + i + 1) * 128],
                                    XP[:, dc, sh * 512 : (sh + 1) * 512],
                                    start=(dc == 0),
                                    stop=(dc == EC - 1),
                                )
                        for i in range(4):
                            ec = ec0 + i
                            nc.vector.tensor_scalar(
                                out=QT[:, ec, sh * 512 : (sh + 1) * 512],
                                in0=pss[i][:, :],
                                scalar1=inv,
                                scalar2=BQ[:, ec : ec + 1],
                                op0=mybir.AluOpType.mult,
                                op1=mybir.AluOpType.add,
                            )

                # readback both parity arrays, block by block ascending so
                # early (narrow) slots unblock first; E on sync, O on scalar
                for b in range(8):
                    nc.sync.dma_start(
                        out=KTE[:, :, b * 128 : (b + 1) * 128],
                        in_=bounce_out[0, :, :, b * 128 : (b + 1) * 128],
                    )
                    nc.scalar.dma_start(
                        out=KTO[:, :, b * 128 : (b + 1) * 128],
                        in_=bounce_out[1, :, :, b * 128 : (b + 1) * 128],
                    )

            # ---- phase B + C: attention + output projection ----
            with (
                tc.tile_pool(name="wo", bufs=1) as wo_pool,
                tc.tile_pool(name="att", bufs=5) as att_pool,
                tc.tile_pool(name="attT", bufs=2) as attT_pool,
                tc.tile_pool(name="ctx", bufs=1) as ctx_pool,
                tc.tile_pool(name="stat", bufs=1) as stat_pool,
                tc.tile_pool(name="sc_psum", bufs=3, space="PSUM") as sc_psum,
                tc.tile_pool(name="mm_psum", bufs=2, space="PSUM") as mm_psum,
                tc.tile_pool(name="outbuf", bufs=2) as out_pool,
            ):
                WO = wo_pool.tile([128, EC, D], BF16, tag="WO")
                for dc in range(EC):
                    nc.sync.dma_start(out=WO[:, dc, :], in_=woT[:, dc, :])
                CTXT = ctx_pool.tile([128, EC, 1024], BF16, tag="CTXT")
                LSUM = stat_pool.tile([128, 2 * NSLOT], F32, tag="LS")
                LTOT = stat_pool.tile([128, NSLOT], F32, tag="LT")

                def out_proj(slot):
                    OUTS = out_pool.tile([128, D], F32, tag="outs")
                    for eh in range(2):
                        ps = mm_psum.tile([128, 512], F32, tag="mm")
                        for dc in range(EC):
                            nc.tensor.matmul(
                                ps[:, :],
                                CTXT[:, dc, slot * 128 : (slot + 1) * 128],
                                WO[:, dc, eh * 512 : (eh + 1) * 512],
                                start=(dc == 0),
                                stop=(dc == EC - 1),
                            )
                        nc.vector.scalar_tensor_tensor(
                            out=OUTS[:, eh * 512 : (eh + 1) * 512],
                            in0=ps[:, :],
                            scalar=RL[:, slot : slot + 1],
                            in1=BOF[:, eh * 512 : (eh + 1) * 512],
                            op0=mybir.AluOpType.mult,
                            op1=mybir.AluOpType.add,
                        )
                        nc.scalar.dma_start(
                            out=out_d[
                                slot * 128 : (slot + 1) * 128,
                                eh * 512 : (eh + 1) * 512,
                            ],
                            in_=OUTS[:, eh * 512 : (eh + 1) * 512],
                        )

                for g in range(2):
                    # ATT_T rows 0..7: E-compact blocks, rows 8..15: O-compact
                    ATT_T = attT_pool.tile([128, NB, 512], BF16, tag="attT")
                    # widest slot first: its long tail overlaps narrower slots
                    for j in (3, 2, 1, 0):
                        slot = g * 4 + j
                        W = 128 * (slot + 1)  # per-parity width
                        ATT = att_pool.tile([128, S], BF16, tag="att")

                        scE = sc_psum.tile([128, 1024], F32, tag="sc")
                        scO = sc_psum.tile([128, 1024], F32, tag="sc")
                        # ec-outer; E+O chunks share the per-ec LDWEIGHTS
                        for ec in range(EC):
                            for c0 in range(0, W, 512):
                                cw = min(512, W - c0)
                                nc.tensor.matmul(
                                    scE[:, c0 : c0 + cw],
                                    QT[:, ec, slot * 128 : (slot + 1) * 128],
                                    KTE[:, ec, c0 : c0 + cw],
                                    start=(ec == 0),
                                    stop=(ec == EC - 1),
                                )
                            for c0 in range(0, W, 512):
                                cw = min(512, W - c0)
                                nc.tensor.matmul(
                                    scO[:, c0 : c0 + cw],
                                    QT[:, ec, slot * 128 : (slot + 1) * 128],
                                    KTO[:, ec, c0 : c0 + cw],
                                    start=(ec == 0),
                                    stop=(ec == EC - 1),
                                )
                        nc.vector.tensor_tensor(
                            out=scE[:, W - 128 : W],
                            in0=scE[:, W - 128 : W],
                            in1=MASK[:, slot, 0:128],
                            op=mybir.AluOpType.add,
                        )
                        nc.vector.tensor_tensor(
                            out=scO[:, W - 128 : W],
                            in0=scO[:, W - 128 : W],
                            in1=MASK[:, slot, 128:256],
                            op=mybir.AluOpType.add,
                        )
                        # no max-subtraction: scores ~ N(0,1), exp is safe;
                        # each parity's exp+transpose fires independently
                        nc.scalar.activation(
                            ATT[:, 0:W],
                            scE[:, :W],
                            mybir.ActivationFunctionType.Exp,
                            bias=0.0,
                            scale=1.0,
                            accum_out=LSUM[:, 2 * slot : 2 * slot + 1],
                        )
                        nc.sync.dma_start_transpose(
                            ATT_T[:, 0 : W // 128, j * 128 : (j + 1) * 128],
                            ATT[:, 0:W],
                        )
                        nc.scalar.activation(
                            ATT[:, 1024 : 1024 + W],
                            scO[:, :W],
                            mybir.ActivationFunctionType.Exp,
                            bias=0.0,
                            scale=1.0,
                            accum_out=LSUM[:, 2 * slot + 1 : 2 * slot + 2],
                        )
                        nc.sync.dma_start_transpose(
                            ATT_T[:, 8 : 8 + W // 128, j * 128 : (j + 1) * 128],
                            ATT[:, 1024 : 1024 + W],
                        )
                        nc.vector.tensor_tensor(
                            out=LTOT[:, slot : slot + 1],
                            in0=LSUM[:, 2 * slot : 2 * slot + 1],
                            in1=LSUM[:, 2 * slot + 1 : 2 * slot + 2],
                            op=mybir.AluOpType.add,
                        )
                        nc.vector.reciprocal(
                            RL[:, slot : slot + 1], LTOT[:, slot : slot + 1]
                        )

                    # attnV: E pass then O pass, one continued accumulation;
                    # compact ragged: slot j covers parity blocks [0..g*4+j]
                    ntc = g * 4 + 4
                    for dc in range(EC):
                        ps = mm_psum.tile([128, 512], F32, tag="mm")
                        for pi, base in ((0, 0), (1, 8)):
                            for tcn in range(ntc):
                                scol = max(0, tcn - g * 4) * 128
                                nc.tensor.matmul(
                                    ps[:, scol:512],
                                    V[:, base + tcn, dc * 128 : (dc + 1) * 128],
                                    ATT_T[:, base + tcn, scol:512],
                                    start=(pi == 0 and tcn == 0),
                                    stop=(pi == 1 and tcn == ntc - 1),
                                )
                        nc.vector.tensor_copy(
                            CTXT[:, dc, g * 512 : (g + 1) * 512], ps[:, :]
                        )
                    for j in range(4):
                        out_proj(g * 4 + j)

    nc.compile()
    return nc


def _core_blocks(core):
    parity = core % 2  # even core (pair rank 0) -> even blocks
    return [2 * s + parity for s in range(NSLOT)]


def _make_in_maps(x, Wq, bq, Wk, bk, Wv, bv, Wo, bo):
    bf = ml_dtypes.bfloat16

    def wt_layout(W):
        return np.ascontiguousarray(
            W.T.astype(bf).reshape(EC, 128, D).transpose(1, 0, 2)
        )

    def xT_layout(xrows, n):
        return np.ascontiguousarray(
            xrows.T.astype(bf).reshape(EC, 128, n).transpose(1, 0, 2)
        )

    wq_l, wk_l, wv_l, wo_l = (wt_layout(W) for W in (Wq, Wk, Wv, Wo))
    bq_l = np.ascontiguousarray(bq.reshape(EC, 128).T.astype(np.float32))
    bk_l = np.ascontiguousarray(bk.reshape(EC, 128).T.astype(np.float32))
    bv_l = np.ascontiguousarray(bv.reshape(1, D).astype(np.float32))
    bo_l = np.ascontiguousarray(bo.reshape(1, D).astype(np.float32))

    in_maps = []
    for core in range(8):
        b = core // 2
        p = core % 2
        blocks = _core_blocks(core)
        xb = np.asarray(x[b], dtype=np.float32)
        # parity-compact x: own blocks (used for Q and K-own)
        xp = np.concatenate([xb[bl * 128 : (bl + 1) * 128] for bl in blocks], axis=0)
        # E-compact | O-compact full x (used for V)
        xeo = np.concatenate(
            [xb[bl * 128 : (bl + 1) * 128] for bl in range(0, NB, 2)]
            + [xb[bl * 128 : (bl + 1) * 128] for bl in range(1, NB, 2)],
            axis=0,
        )
        # mask: [:, s, 0:128] on E-compact block s, [:, s, 128:256] on O s.
        # diagonal lives in the own-parity array; the other parity's block s
        # is fully open for odd cores, fully masked for even cores.
        mask = np.zeros((128, NSLOT, 256), np.float32)
        r = np.arange(128)[:, None]
        o = np.arange(128)[None, :]
        diag = np.where(o <= r, 0.0, NEG)
        for s_i in range(NSLOT):
            if p == 0:
                mask[:, s_i, 0:128] = diag
                mask[:, s_i, 128:256] = NEG
            else:
                mask[:, s_i, 0:128] = 0.0
                mask[:, s_i, 128:256] = diag
        in_maps.append(
            {
                "xpT": xT_layout(xp, 1024),
                "xeoT": xT_layout(xeo, S),
                "wqT": wq_l,
                "wkT": wk_l,
                "wvT": wv_l,
                "woT": wo_l,
                "bq": bq_l,
                "bk": bk_l,
                "bv": bv_l,
                "bo": bo_l,
                "mask": mask,
            }
        )
    return in_maps


def _run(inputs, trace=False):
    global _compiled
    if _compiled is None:
        _compiled = _build()
    nc = _compiled
    in_maps = _make_in_maps(**inputs)
    res = run_bass_kernel_spmd(nc, in_maps, core_ids=list(range(8)), trace=trace)
    out = np.zeros((B, S, D), np.float32)
    for core in range(8):
        b = core // 2
        o = res.results[core]["out"]
        for s_i, bl in enumerate(_core_blocks(core)):
            out[b, bl * 128 : (bl + 1) * 128, :] = o[s_i * 128 : (s_i + 1) * 128, :]
    return out, res


def kernel(**inputs):
    out, _ = _run(inputs, trace=False)
    return out


# revision 16
# speedup vs baseline: 1.0737x; 1.0670x over previous
"""Distributed single-head causal attention for Trainium2 (8 NeuronCores).

Problem: x:[4,2048,1024] f32, Wq/Wk/Wv/Wo:[1024,1024], b*:[1024]
  q = x@Wq.T+bq; k = x@Wk.T+bk; v = x@Wv.T+bv
  scores = (q@k.T)/sqrt(1024) causal-masked; out = softmax(scores)@v @Wo.T + bo

Sharding (data-parallel pairs, K exchanged by block parity, strict-SPMD):
  8 cores = 4 batches x 2 cores/batch. The 16 query blocks (128 rows) of a
  batch split by parity: even core takes even blocks, odd core odd blocks.
  Every core runs 8 "slots" with the compile-time schedule T_s = 256*(s+1)
  over keys in LOGICAL order -> identical instruction streams, balanced
  causal work, causality via a host-built tail mask.

K is projected only for the core's own parity blocks (the parity-compact
x input is shared with Q proj) and exchanged within the pair by a 2MB
DRAM-bounce AllGather (measured ~26us start + ~100GB/s; hides behind the
V + Q projections). AllGather output is rank-ordered, so ccout[0] is
K-even and ccout[1] is K-odd on BOTH cores; the readback's 16 block DMAs
interleave the halves straight back into logical key order - no rank
awareness anywhere. V is projected fully on-core from an E|O-compact x
copy, with a static block permutation writing logical order. No
max-subtraction in softmax (scores ~N(0,1), exp overflow-safe);
normalization (1/l) folds into the output projection.

Per-core PE: K-own 65536cyc + V-full 131072 + Q 65536 + scores 73728 +
attnV 73728 + out 65536 = 475k cycles (~198us @2.4GHz).
"""

import sys

if "/opt/trn_rl_repo" not in sys.path:
    sys.path.insert(0, "/opt/trn_rl_repo")

import numpy as np
import ml_dtypes

import concourse.bass as bass
import concourse.mybir as mybir
from concourse import bacc
from concourse.bass_utils import run_bass_kernel_spmd
from concourse.tile import TileContext

B, S, D = 4, 2048, 1024
NB = S // 128
NSLOT = 8
EC = D // 128
F32 = mybir.dt.float32
BF16 = mybir.dt.bfloat16
NEG = -1.0e9
GROUPS = [[0, 1], [2, 3], [4, 5], [6, 7]]

_compiled = None


def _slot_T(s):
    return 256 * (s + 1)


def _build():
    nc = bacc.Bacc("TRN2", target_bir_lowering=False, debug=False, num_devices=8)

    # xpT: parity-compact x (the core's own 8 blocks) - feeds Q proj AND K-own
    xpT = nc.dram_tensor("xpT", [128, EC, 1024], BF16, kind="ExternalInput")
    # xeoT: full x, E-compact | O-compact arrangement - feeds V proj
    xeoT = nc.dram_tensor("xeoT", [128, EC, S], BF16, kind="ExternalInput")
    wqT = nc.dram_tensor("wqT", [128, EC, D], BF16, kind="ExternalInput")
    wkT = nc.dram_tensor("wkT", [128, EC, D], BF16, kind="ExternalInput")
    wvT = nc.dram_tensor("wvT", [128, EC, D], BF16, kind="ExternalInput")
    woT = nc.dram_tensor("woT", [128, EC, D], BF16, kind="ExternalInput")
    bq_d = nc.dram_tensor("bq", [128, EC], F32, kind="ExternalInput")
    bk_d = nc.dram_tensor("bk", [128, EC], F32, kind="ExternalInput")
    bv_d = nc.dram_tensor("bv", [1, D], F32, kind="ExternalInput")
    bo_d = nc.dram_tensor("bo", [1, D], F32, kind="ExternalInput")
    mask_d = nc.dram_tensor("mask", [128, NSLOT, 256], F32, kind="ExternalInput")
    out_d = nc.dram_tensor("out", [NSLOT * 128, D], F32, kind="ExternalOutput")

    inv = 1.0 / 32.0

    with TileContext(nc) as tc:
        with (
            tc.tile_pool(name="persist", bufs=1) as persist,
            tc.tile_pool(name="small", bufs=1) as small,
            tc.tile_pool(name="dram", bufs=1, space="DRAM") as dram,
        ):
            QT = persist.tile([128, EC, 1024], BF16, tag="QT")
            KT = persist.tile([128, EC, S], BF16, tag="KT")  # logical order
            V = persist.tile([128, NB, D], BF16, tag="V")  # logical order
            MASK = small.tile([128, NSLOT, 256], F32, tag="MASK")
            BQ = small.tile([128, EC], F32, tag="BQ")
            BK = small.tile([128, EC], F32, tag="BK")
            RL = small.tile([128, NSLOT], F32, tag="RL")
            BOF = small.tile([128, D], F32, tag="BOF")

            bounce_in = dram.tile([128, EC, 1024], BF16, name="bnc_in")
            bounce_out = dram.tile([2, 128, EC, 1024], BF16, name="bnc_out")

            # ---- phase A: K-own proj -> bounce -> pair AllGather;
            #      V-full + Q proj overlap the collective ----
            with (
                tc.tile_pool(name="xin", bufs=1) as xin,
                tc.tile_pool(name="wts", bufs=1) as wts,
                tc.tile_pool(name="ko", bufs=1) as ko_pool,
                tc.tile_pool(name="pa_psum", bufs=8, space="PSUM") as pa_psum,
            ):
                XP = xin.tile([128, EC, 1024], BF16, tag="XP")
                WQ = wts.tile([128, EC, D], BF16, tag="WQ")
                WK = wts.tile([128, EC, D], BF16, tag="WK")
                WV = wts.tile([128, EC, D], BF16, tag="WV")
                BVF = xin.tile([128, D], F32, tag="BVF")
                KTOWN = ko_pool.tile([128, EC, 1024], BF16, tag="KTOWN")

                bv_row = small.tile([1, D], F32, tag="bv_row")
                nc.sync.dma_start(out=bv_row[:, :], in_=bv_d[:, :])
                nc.gpsimd.partition_broadcast(BVF[:, :], bv_row[:1, :])
                nc.sync.dma_start(out=BK[:, :], in_=bk_d[:, :])
                # WK fully first (gates the first K-own wave), then XP, then
                # WV - every tensor's chunks split across both queues so the
                # first waves run at combined bandwidth
                for dc in range(EC):
                    eng = nc.scalar if dc % 2 == 0 else nc.sync
                    eng.dma_start(out=WK[:, dc, :], in_=wkT[:, dc, :])
                for dc in range(EC):
                    eng = nc.scalar if dc % 2 == 1 else nc.sync
                    eng.dma_start(out=XP[:, dc, :], in_=xpT[:, dc, :])
                for dc in range(EC):
                    eng = nc.scalar if dc % 2 == 0 else nc.sync
                    eng.dma_start(out=WV[:, dc, :], in_=wvT[:, dc, :])

                # K-own: 2 x 512-col strips of the parity-compact x
                for th in range(2):
                    for wv2 in range(2):
                        ec0 = 4 * wv2
                        pss = [
                            pa_psum.tile(
                                [128, 512], F32, tag="pa", name=f"pak{th}_{wv2}_{i}"
                            )
                            for i in range(4)
                        ]
                        for dc in range(EC):
                            for i in range(4):
                                nc.tensor.matmul(
                                    pss[i][:, :],
                                    WK[:, dc, (ec0 + i) * 128 : (ec0 + i + 1) * 128],
                                    XP[:, dc, th * 512 : (th + 1) * 512],
                                    start=(dc == 0),
                                    stop=(dc == EC - 1),
                                )
                        for i in range(4):
                            ec = ec0 + i
                            nc.vector.tensor_scalar(
                                out=KTOWN[:, ec, th * 512 : (th + 1) * 512],
                                in0=pss[i][:, :],
                                scalar1=BK[:, ec : ec + 1],
                                scalar2=None,
                                op0=mybir.AluOpType.add,
                            )
                    # ship this strip to the bounce buffer as soon as done
                    nc.scalar.dma_start(
                        out=bounce_in[:, :, th * 512 : (th + 1) * 512],
                        in_=KTOWN[:, :, th * 512 : (th + 1) * 512],
                    )

                # pair exchange: ccout[0] = K-even, ccout[1] = K-odd on BOTH
                # cores (AllGather output is rank-ordered) -> rank-free readback
                nc.gpsimd.collective_compute(
                    "AllGather",
                    mybir.AluOpType.bypass,
                    replica_groups=GROUPS,
                    ins=[bounce_in.opt()],
                    outs=[bounce_out.opt()],
                )

                for dc in range(EC):
                    nc.scalar.dma_start(out=WQ[:, dc, :], in_=wqT[:, dc, :])

                # V-full from the E|O-compact x copy; targets permuted so V
                # lands in LOGICAL block order (static map, same on all cores)
                def perm(a):
                    return 2 * a if a < 8 else 2 * (a - 8) + 1

                with tc.tile_pool(name="xeo", bufs=2) as xeo_pool:
                    for th in range(4):
                        if th == 2:
                            bq_raw = small.tile([128, EC], F32, tag="bq_raw")
                            nc.sync.dma_start(out=bq_raw[:, :], in_=bq_d[:, :])
                            nc.scalar.mul(BQ[:, :], bq_raw[:, :], inv)
                            nc.sync.dma_start(out=MASK[:, :, :], in_=mask_d[:, :, :])
                            bo_row = small.tile([1, D], F32, tag="bo_row")
                            nc.sync.dma_start(out=bo_row[:, :], in_=bo_d[:, :])
                            nc.gpsimd.partition_broadcast(BOF[:, :], bo_row[:1, :])
                        XEs = xeo_pool.tile(
                            [128, EC, 512], BF16, tag="xeo", name=f"xeo{th}"
                        )
                        for dc in range(EC):
                            nc.sync.dma_start(
                                out=XEs[:, dc, :],
                                in_=xeoT[:, dc, th * 512 : (th + 1) * 512],
                            )
                        for wv2 in range(2):
                            tb0 = 4 * th + 2 * wv2
                            pss = [
                                pa_psum.tile(
                                    [128, 512], F32, tag="pa",
                                    name=f"pavf{th}_{wv2}_{i}",
                                )
                                for i in range(4)
                            ]
                            for dc in range(EC):
                                for i, (tb, dh) in enumerate(
                                    [(tb0, 0), (tb0, 1), (tb0 + 1, 0), (tb0 + 1, 1)]
                                ):
                                    nc.tensor.matmul(
                                        pss[i][:, :],
                                        XEs[:, dc, (tb - 4 * th) * 128 : (tb - 4 * th + 1) * 128],
                                        WV[:, dc, dh * 512 : (dh + 1) * 512],
                                        start=(dc == 0),
                                        stop=(dc == EC - 1),
                                    )
                            for i, (tb, dh) in enumerate(
                                [(tb0, 0), (tb0, 1), (tb0 + 1, 0), (tb0 + 1, 1)]
                            ):
                                nc.vector.tensor_tensor(
                                    out=V[:, perm(tb), dh * 512 : (dh + 1) * 512],
                                    in0=pss[i][:, :],
                                    in1=BVF[:, dh * 512 : (dh + 1) * 512],
                                    op=mybir.AluOpType.add,
                                )

                # QT (x 1/32, +bq/32) from the same parity-compact x
                for sh in range(2):
                    for w in range(2):
                        ec0 = 4 * w
                        pss = [
                            pa_psum.tile(
                                [128, 512], F32, tag="pa", name=f"paq{sh}_{w}_{i}"
                            )
                            for i in range(4)
                        ]
                        for dc in range(EC):
                            for i in range(4):
                                nc.tensor.matmul(
                                    pss[i][:, :],
                                    WQ[:, dc, (ec0 + i) * 128 : (ec0 + i + 1) * 128],
                                    XP[:, dc, sh * 512 : (sh + 1) * 512],
                                    start=(dc == 0),
                                    stop=(dc == EC - 1),
                                )
                        for i in range(4):
                            ec = ec0 + i
                            nc.vector.tensor_scalar(
                                out=QT[:, ec, sh * 512 : (sh + 1) * 512],
                                in0=pss[i][:, :],
                                scalar1=inv,
                                scalar2=BQ[:, ec : ec + 1],
                                op0=mybir.AluOpType.mult,
                                op1=mybir.AluOpType.add,
                            )

                # readback: interleave the two parity halves straight into
                # logical key order; blocks ascending so narrow slots
                # unblock first; halves alternate across the two queues
                for b in range(8):
                    nc.sync.dma_start(
                        out=KT[:, :, (2 * b) * 128 : (2 * b + 1) * 128],
                        in_=bounce_out[0, :, :, b * 128 : (b + 1) * 128],
                    )
                    nc.scalar.dma_start(
                        out=KT[:, :, (2 * b + 1) * 128 : (2 * b + 2) * 128],
                        in_=bounce_out[1, :, :, b * 128 : (b + 1) * 128],
                    )

            # ---- phase B + C: attention + output projection ----
            with (
                tc.tile_pool(name="wo", bufs=1) as wo_pool,
                tc.tile_pool(name="att", bufs=5) as att_pool,
                tc.tile_pool(name="attT", bufs=2) as attT_pool,
                tc.tile_pool(name="ctx", bufs=1) as ctx_pool,
                tc.tile_pool(name="stat", bufs=1) as stat_pool,
                tc.tile_pool(name="sc_psum", bufs=3, space="PSUM") as sc_psum,
                tc.tile_pool(name="mm_psum", bufs=2, space="PSUM") as mm_psum,
                tc.tile_pool(name="outbuf", bufs=2) as out_pool,
            ):
                WO = wo_pool.tile([128, EC, D], BF16, tag="WO")
                for dc in range(EC):
                    nc.sync.dma_start(out=WO[:, dc, :], in_=woT[:, dc, :])
                CTXT = ctx_pool.tile([128, EC, 1024], BF16, tag="CTXT")
                LSUM = stat_pool.tile([128, 2 * NSLOT], F32, tag="LS")
                LTOT = stat_pool.tile([128, NSLOT], F32, tag="LT")

                def out_proj(slot):
                    OUTS = out_pool.tile([128, D], F32, tag="outs")
                    for eh in range(2):
                        ps = mm_psum.tile([128, 512], F32, tag="mm")
                        for dc in range(EC):
                            nc.tensor.matmul(
                                ps[:, :],
                                CTXT[:, dc, slot * 128 : (slot + 1) * 128],
                                WO[:, dc, eh * 512 : (eh + 1) * 512],
                                start=(dc == 0),
                                stop=(dc == EC - 1),
                            )
                        nc.vector.scalar_tensor_tensor(
                            out=OUTS[:, eh * 512 : (eh + 1) * 512],
                            in0=ps[:, :],
                            scalar=RL[:, slot : slot + 1],
                            in1=BOF[:, eh * 512 : (eh + 1) * 512],
                            op0=mybir.AluOpType.mult,
                            op1=mybir.AluOpType.add,
                        )
                        # per-half store on the scalar queue (idle in phase B)
                        nc.scalar.dma_start(
                            out=out_d[
                                slot * 128 : (slot + 1) * 128,
                                eh * 512 : (eh + 1) * 512,
                            ],
                            in_=OUTS[:, eh * 512 : (eh + 1) * 512],
                        )

                for g in range(2):
                    ATT_T = attT_pool.tile([128, NB, 512], BF16, tag="attT")
                    # widest slot first: its long scores+softmax+transpose
                    # chain overlaps the narrower slots; the group's attnV
                    # gate becomes the narrowest slot's short tail.
                    for j in (3, 2, 1, 0):
                        slot = g * 4 + j
                        T = _slot_T(slot)
                        ATT = att_pool.tile([128, S], BF16, tag="att")

                        nparts = (T + 1023) // 1024
                        parts = []
                        for p in range(nparts):
                            w = min(1024, T - p * 1024)
                            sc = sc_psum.tile([128, 1024], F32, tag="sc")
                            parts.append((sc, w))
                        # ec-outer: one LDWEIGHTS per ec covers the whole row
                        for ec in range(EC):
                            for p, (sc, w) in enumerate(parts):
                                for c0 in range(0, w, 512):
                                    cw = min(512, w - c0)
                                    a0 = p * 1024 + c0
                                    nc.tensor.matmul(
                                        sc[:, c0 : c0 + cw],
                                        QT[:, ec, slot * 128 : (slot + 1) * 128],
                                        KT[:, ec, a0 : a0 + cw],
                                        start=(ec == 0),
                                        stop=(ec == EC - 1),
                                    )
                        lsc, lw = parts[-1]
                        nc.vector.tensor_tensor(
                            out=lsc[:, lw - 256 : lw],
                            in0=lsc[:, lw - 256 : lw],
                            in1=MASK[:, slot, :],
                            op=mybir.AluOpType.add,
                        )
                        # no max-subtraction: scores ~ N(0,1) (|s|<~6), so
                        # exp() is overflow-safe and each part's exp +
                        # transpose fires as soon as that part's scores land
                        for p, (sc, w) in enumerate(parts):
                            nc.scalar.activation(
                                ATT[:, p * 1024 : p * 1024 + w],
                                sc[:, :w],
                                mybir.ActivationFunctionType.Exp,
                                bias=0.0,
                                scale=1.0,
                                accum_out=LSUM[:, 2 * slot + p : 2 * slot + p + 1],
                            )
                            nc.sync.dma_start_transpose(
                                ATT_T[:, p * 8 : p * 8 + w // 128, j * 128 : (j + 1) * 128],
                                ATT[:, p * 1024 : p * 1024 + w],
                            )
                        if nparts == 2:
                            nc.vector.tensor_tensor(
                                out=LTOT[:, slot : slot + 1],
                                in0=LSUM[:, 2 * slot : 2 * slot + 1],
                                in1=LSUM[:, 2 * slot + 1 : 2 * slot + 2],
                                op=mybir.AluOpType.add,
                            )
                            nc.vector.reciprocal(
                                RL[:, slot : slot + 1], LTOT[:, slot : slot + 1]
                            )
                        else:
                            nc.vector.reciprocal(
                                RL[:, slot : slot + 1],
                                LSUM[:, 2 * slot : 2 * slot + 1],
                            )

                    ntg = _slot_T(g * 4 + 3) // 128
                    for dc in range(EC):
                        ps = mm_psum.tile([128, 512], F32, tag="mm")
                        for tcn in range(ntg):
                            jmin = 0
                            for jj in range(4):
                                if 256 * (g * 4 + jj + 1) >= 128 * (tcn + 1):
                                    jmin = jj
                                    break
                            scol = jmin * 128
                            nc.tensor.matmul(
                                ps[:, scol:512],
                                V[:, tcn, dc * 128 : (dc + 1) * 128],
                                ATT_T[:, tcn, scol:512],
                                start=(tcn == 0),
                                stop=(tcn == ntg - 1),
                            )
                        nc.vector.tensor_copy(
                            CTXT[:, dc, g * 512 : (g + 1) * 512], ps[:, :]
                        )
                    for j in range(4):
                        out_proj(g * 4 + j)

    nc.compile()
    return nc


def _core_blocks(core):
    parity = core % 2  # even core (pair rank 0) -> even blocks
    return [2 * s + parity for s in range(NSLOT)]


def _make_in_maps(x, Wq, bq, Wk, bk, Wv, bv, Wo, bo):
    bf = ml_dtypes.bfloat16

    def wt_layout(W):
        return np.ascontiguousarray(
            W.T.astype(bf).reshape(EC, 128, D).transpose(1, 0, 2)
        )

    def xT_layout(xrows, n):
        return np.ascontiguousarray(
            xrows.T.astype(bf).reshape(EC, 128, n).transpose(1, 0, 2)
        )

    wq_l, wk_l, wv_l, wo_l = (wt_layout(W) for W in (Wq, Wk, Wv, Wo))
    bq_l = np.ascontiguousarray(bq.reshape(EC, 128).T.astype(np.float32))
    bk_l = np.ascontiguousarray(bk.reshape(EC, 128).T.astype(np.float32))
    bv_l = np.ascontiguousarray(bv.reshape(1, D).astype(np.float32))
    bo_l = np.ascontiguousarray(bo.reshape(1, D).astype(np.float32))

    in_maps = []
    for core in range(8):
        b = core // 2
        blocks = _core_blocks(core)
        xb = np.asarray(x[b], dtype=np.float32)
        # parity-compact x: own blocks (used for Q and K-own)
        xp = np.concatenate([xb[bl * 128 : (bl + 1) * 128] for bl in blocks], axis=0)
        # E-compact | O-compact full x (used for V)
        xeo = np.concatenate(
            [xb[bl * 128 : (bl + 1) * 128] for bl in range(0, NB, 2)]
            + [xb[bl * 128 : (bl + 1) * 128] for bl in range(1, NB, 2)],
            axis=0,
        )
        # tail mask over the last 256 logical key columns of each slot
        mask = np.zeros((128, NSLOT, 256), np.float32)
        r = np.arange(128)[:, None]
        jj = np.arange(256)[None, :]
        for s_i, bl in enumerate(blocks):
            lim = bl * 128 + r
            t_idx = 256 * s_i + jj
            mask[:, s_i, :] = np.where(t_idx <= lim, 0.0, NEG)
        in_maps.append(
            {
                "xpT": xT_layout(xp, 1024),
                "xeoT": xT_layout(xeo, S),
                "wqT": wq_l,
                "wkT": wk_l,
                "wvT": wv_l,
                "woT": wo_l,
                "bq": bq_l,
                "bk": bk_l,
                "bv": bv_l,
                "bo": bo_l,
                "mask": mask,
            }
        )
    return in_maps


def _run(inputs, trace=False):
    global _compiled
    if _compiled is None:
        _compiled = _build()
    nc = _compiled
    in_maps = _make_in_maps(**inputs)
    res = run_bass_kernel_spmd(nc, in_maps, core_ids=list(range(8)), trace=trace)
    out = np.zeros((B, S, D), np.float32)
    for core in range(8):
        b = core // 2
        o = res.results[core]["out"]
        for s_i, bl in enumerate(_core_blocks(core)):
            out[b, bl * 128 : (bl + 1) * 128, :] = o[s_i * 128 : (s_i + 1) * 128, :]
    return out, res


def kernel(**inputs):
    out, _ = _run(inputs, trace=False)
    return out


# revision 21
# speedup vs baseline: 1.0925x; 1.0175x over previous
"""Distributed single-head causal attention for Trainium2 (8 NeuronCores).

Problem: x:[4,2048,1024] f32, Wq/Wk/Wv/Wo:[1024,1024], b*:[1024]
  q = x@Wq.T+bq; k = x@Wk.T+bk; v = x@Wv.T+bv
  scores = (q@k.T)/sqrt(1024) causal-masked; out = softmax(scores)@v @Wo.T + bo

Sharding (data-parallel pairs, K exchanged by block parity, strict-SPMD):
  8 cores = 4 batches x 2 cores/batch. The 16 query blocks (128 rows) of a
  batch split by parity: even core takes even blocks, odd core odd blocks.
  Every core runs 8 "slots" with the compile-time schedule T_s = 256*(s+1)
  over keys in LOGICAL order -> identical instruction streams, balanced
  causal work, causality via a host-built tail mask.

K is projected only for the core's own parity blocks (the parity-compact
x input is shared with Q proj) and exchanged within the pair by a 2MB
DRAM-bounce AllGather (measured ~26us start + ~100GB/s; hides behind the
V + Q projections). AllGather output is rank-ordered, so ccout[0] is
K-even and ccout[1] is K-odd on BOTH cores; the readback's 16 block DMAs
interleave the halves straight back into logical key order - no rank
awareness anywhere. V is projected fully on-core from an E|O-compact x
copy, with a static block permutation writing logical order. No
max-subtraction in softmax (scores ~N(0,1), exp overflow-safe);
normalization (1/l) folds into the output projection.

Per-core PE: K-own 65536cyc + V-full 131072 + Q 65536 + scores 73728 +
attnV 73728 + out 65536 = 475k cycles (~198us @2.4GHz).
"""

import sys

if "/opt/trn_rl_repo" not in sys.path:
    sys.path.insert(0, "/opt/trn_rl_repo")

import numpy as np
import ml_dtypes

import concourse.bass as bass
import concourse.mybir as mybir
from concourse import bacc
from concourse.bass_utils import run_bass_kernel_spmd
from concourse.tile import TileContext

B, S, D = 4, 2048, 1024
NB = S // 128
NSLOT = 8
EC = D // 128
F32 = mybir.dt.float32
BF16 = mybir.dt.bfloat16
NEG = -1.0e9
GROUPS = [[0, 1], [2, 3], [4, 5], [6, 7]]

_compiled = None


def _slot_T(s):
    return 256 * (s + 1)


def _build():
    nc = bacc.Bacc("TRN2", target_bir_lowering=False, debug=False, num_devices=8)

    # xpT: parity-compact x (the core's own 8 blocks) - feeds Q proj AND K-own
    xpT = nc.dram_tensor("xpT", [128, EC, 1024], BF16, kind="ExternalInput")
    # xeoT: full x, E-compact | O-compact arrangement - feeds V proj
    xeoT = nc.dram_tensor("xeoT", [128, EC, S], BF16, kind="ExternalInput")
    wqT = nc.dram_tensor("wqT", [128, EC, D], BF16, kind="ExternalInput")
    wkT = nc.dram_tensor("wkT", [128, EC, D], BF16, kind="ExternalInput")
    wvT = nc.dram_tensor("wvT", [128, EC, D], BF16, kind="ExternalInput")
    woT = nc.dram_tensor("woT", [128, EC, D], BF16, kind="ExternalInput")
    bq_d = nc.dram_tensor("bq", [128, EC], F32, kind="ExternalInput")
    bk_d = nc.dram_tensor("bk", [128, EC], F32, kind="ExternalInput")
    bv_d = nc.dram_tensor("bv", [1, D], F32, kind="ExternalInput")
    bo_d = nc.dram_tensor("bo", [1, D], F32, kind="ExternalInput")
    mask_d = nc.dram_tensor("mask", [128, NSLOT, 256], F32, kind="ExternalInput")
    out_d = nc.dram_tensor("out", [NSLOT * 128, D], F32, kind="ExternalOutput")

    inv = 1.0 / 32.0

    with TileContext(nc) as tc:
        with (
            tc.tile_pool(name="persist", bufs=1) as persist,
            tc.tile_pool(name="small", bufs=1) as small,
            tc.tile_pool(name="dram", bufs=1, space="DRAM") as dram,
        ):
            QT = persist.tile([128, EC, 1024], BF16, tag="QT")
            KT = persist.tile([128, EC, S], BF16, tag="KT")  # logical order
            V = persist.tile([128, NB, D], BF16, tag="V")  # logical order
            MASK = small.tile([128, NSLOT, 256], F32, tag="MASK")
            BQ = small.tile([128, EC], F32, tag="BQ")
            BK = small.tile([128, EC], F32, tag="BK")
            RL = small.tile([128, NSLOT], F32, tag="RL")
            BOF = small.tile([128, D], F32, tag="BOF")

            bounce_in = dram.tile([128, EC, 1024], BF16, name="bnc_in")
            bounce_out = dram.tile([2, 128, EC, 1024], BF16, name="bnc_out")

            # ---- phase A: K-own proj -> bounce -> pair AllGather;
            #      V-full + Q proj overlap the collective ----
            with (
                tc.tile_pool(name="xin", bufs=1) as xin,
                tc.tile_pool(name="wts", bufs=1) as wts,
                tc.tile_pool(name="ko", bufs=1) as ko_pool,
                # 6 bufs: two PSUM banks stay virgin so phase-B's first
                # scores tile can allocate while V/Q waves still run
                tc.tile_pool(name="pa_psum", bufs=6, space="PSUM") as pa_psum,
            ):
                XP = xin.tile([128, EC, 1024], BF16, tag="XP")
                WQ = wts.tile([128, EC, D], BF16, tag="WQ")
                WK = wts.tile([128, EC, D], BF16, tag="WK")
                WV = wts.tile([128, EC, D], BF16, tag="WV")
                BVF = xin.tile([128, D], F32, tag="BVF")
                KTOWN = ko_pool.tile([128, EC, 1024], BF16, tag="KTOWN")

                bv_row = small.tile([1, D], F32, tag="bv_row")
                nc.sync.dma_start(out=bv_row[:, :], in_=bv_d[:, :])
                nc.gpsimd.partition_broadcast(BVF[:, :], bv_row[:1, :])
                nc.sync.dma_start(out=BK[:, :], in_=bk_d[:, :])
                # interleave WK/XP chunks in the K-wave's consumption order
                # (pair dc arrives together), split across both queues
                for dc in range(EC):
                    a, b = (nc.scalar, nc.sync) if dc % 2 == 0 else (nc.sync, nc.scalar)
                    a.dma_start(out=WK[:, dc, :], in_=wkT[:, dc, :])
                    b.dma_start(out=XP[:, dc, :], in_=xpT[:, dc, :])
                for dc in range(EC):
                    eng = nc.scalar if dc % 2 == 0 else nc.sync
                    eng.dma_start(out=WV[:, dc, :], in_=wvT[:, dc, :])
                # WQ issued pre-compute so it lands before the early Q proj
                for dc in range(EC):
                    eng = nc.scalar if dc % 2 == 1 else nc.sync
                    eng.dma_start(out=WQ[:, dc, :], in_=wqT[:, dc, :])

                # K-own: 2 x 512-col strips of the parity-compact x
                for th in range(2):
                    for wv2 in range(2):
                        ec0 = 4 * wv2
                        pss = [
                            pa_psum.tile(
                                [128, 512], F32, tag="pa", name=f"pak{th}_{wv2}_{i}"
                            )
                            for i in range(4)
                        ]
                        for dc in range(EC):
                            for i in range(4):
                                nc.tensor.matmul(
                                    pss[i][:, :],
                                    WK[:, dc, (ec0 + i) * 128 : (ec0 + i + 1) * 128],
                                    XP[:, dc, th * 512 : (th + 1) * 512],
                                    start=(dc == 0),
                                    stop=(dc == EC - 1),
                                )
                        for i in range(4):
                            ec = ec0 + i
                            nc.vector.tensor_scalar(
                                out=KTOWN[:, ec, th * 512 : (th + 1) * 512],
                                in0=pss[i][:, :],
                                scalar1=BK[:, ec : ec + 1],
                                scalar2=None,
                                op0=mybir.AluOpType.add,
                            )
                    # ship this strip to the bounce buffer as soon as done
                    nc.scalar.dma_start(
                        out=bounce_in[:, :, th * 512 : (th + 1) * 512],
                        in_=KTOWN[:, :, th * 512 : (th + 1) * 512],
                    )

                # pair exchange: ccout[0] = K-even, ccout[1] = K-odd on BOTH
                # cores (AllGather output is rank-ordered) -> rank-free readback
                nc.gpsimd.collective_compute(
                    "AllGather",
                    mybir.AluOpType.bypass,
                    replica_groups=GROUPS,
                    ins=[bounce_in.opt()],
                    outs=[bounce_out.opt()],
                )

                bq_raw = small.tile([128, EC], F32, tag="bq_raw")
                nc.sync.dma_start(out=bq_raw[:, :], in_=bq_d[:, :])
                nc.scalar.mul(BQ[:, :], bq_raw[:, :], inv)
                nc.sync.dma_start(out=MASK[:, :, :], in_=mask_d[:, :, :])
                bo_row = small.tile([1, D], F32, tag="bo_row")
                nc.sync.dma_start(out=bo_row[:, :], in_=bo_d[:, :])
                nc.gpsimd.partition_broadcast(BOF[:, :], bo_row[:1, :])

                # QT (x 1/32, +bq/32) right after K-own: QT is ready early so
                # the first scores can interleave with the V-projection tail
                for sh in range(2):
                    for w in range(2):
                        ec0 = 4 * w
                        pss = [
                            pa_psum.tile(
                                [128, 512], F32, tag="pa", name=f"paq{sh}_{w}_{i}"
                            )
                            for i in range(4)
                        ]
                        for dc in range(EC):
                            for i in range(4):
                                nc.tensor.matmul(
                                    pss[i][:, :],
                                    WQ[:, dc, (ec0 + i) * 128 : (ec0 + i + 1) * 128],
                                    XP[:, dc, sh * 512 : (sh + 1) * 512],
                                    start=(dc == 0),
                                    stop=(dc == EC - 1),
                                )
                        for i in range(4):
                            ec = ec0 + i
                            nc.vector.tensor_scalar(
                                out=QT[:, ec, sh * 512 : (sh + 1) * 512],
                                in0=pss[i][:, :],
                                scalar1=inv,
                                scalar2=BQ[:, ec : ec + 1],
                                op0=mybir.AluOpType.mult,
                                op1=mybir.AluOpType.add,
                            )

                # V-full from the E|O-compact x copy; targets permuted so V
                # lands in LOGICAL block order (static map, same on all cores).
                # Strip order [0,2,1,3]: logical blocks 0-7 (attnV group 0's
                # needs) complete after two strips.
                def perm(a):
                    return 2 * a if a < 8 else 2 * (a - 8) + 1

                with tc.tile_pool(name="xeo", bufs=2) as xeo_pool:
                    for th in (0, 2, 1, 3):
                        XEs = xeo_pool.tile(
                            [128, EC, 512], BF16, tag="xeo", name=f"xeo{th}"
                        )
                        for dc in range(EC):
                            nc.sync.dma_start(
                                out=XEs[:, dc, :],
                                in_=xeoT[:, dc, th * 512 : (th + 1) * 512],
                            )
                        for wv2 in range(2):
                            tb0 = 4 * th + 2 * wv2
                            pss = [
                                pa_psum.tile(
                                    [128, 512], F32, tag="pa",
                                    name=f"pavf{th}_{wv2}_{i}",
                                )
                                for i in range(4)
                            ]
                            for dc in range(EC):
                                for i, (tb, dh) in enumerate(
                                    [(tb0, 0), (tb0, 1), (tb0 + 1, 0), (tb0 + 1, 1)]
                                ):
                                    nc.tensor.matmul(
                                        pss[i][:, :],
                                        XEs[:, dc, (tb - 4 * th) * 128 : (tb - 4 * th + 1) * 128],
                                        WV[:, dc, dh * 512 : (dh + 1) * 512],
                                        start=(dc == 0),
                                        stop=(dc == EC - 1),
                                    )
                            for i, (tb, dh) in enumerate(
                                [(tb0, 0), (tb0, 1), (tb0 + 1, 0), (tb0 + 1, 1)]
                            ):
                                nc.vector.tensor_tensor(
                                    out=V[:, perm(tb), dh * 512 : (dh + 1) * 512],
                                    in0=pss[i][:, :],
                                    in1=BVF[:, dh * 512 : (dh + 1) * 512],
                                    op=mybir.AluOpType.add,
                                )

                # readback: interleave the two parity halves straight into
                # logical key order; blocks ascending so narrow slots
                # unblock first; halves alternate across the two queues
                for b in range(8):
                    nc.sync.dma_start(
                        out=KT[:, :, (2 * b) * 128 : (2 * b + 1) * 128],
                        in_=bounce_out[0, :, :, b * 128 : (b + 1) * 128],
                    )
                    nc.scalar.dma_start(
                        out=KT[:, :, (2 * b + 1) * 128 : (2 * b + 2) * 128],
                        in_=bounce_out[1, :, :, b * 128 : (b + 1) * 128],
                    )

            # ---- phase B + C: attention + output projection ----
            with (
                tc.tile_pool(name="wo", bufs=1) as wo_pool,
                tc.tile_pool(name="att", bufs=5) as att_pool,
                tc.tile_pool(name="attT", bufs=2) as attT_pool,
                tc.tile_pool(name="ctx", bufs=1) as ctx_pool,
                tc.tile_pool(name="stat", bufs=1) as stat_pool,
                tc.tile_pool(name="sc_psum", bufs=3, space="PSUM") as sc_psum,
                tc.tile_pool(name="mm_psum", bufs=2, space="PSUM") as mm_psum,
                tc.tile_pool(name="outbuf", bufs=2) as out_pool,
            ):
                WO = wo_pool.tile([128, EC, D], BF16, tag="WO")
                for dc in range(EC):
                    nc.sync.dma_start(out=WO[:, dc, :], in_=woT[:, dc, :])
                CTXT = ctx_pool.tile([128, EC, 1024], BF16, tag="CTXT")
                LSUM = stat_pool.tile([128, 2 * NSLOT], F32, tag="LS")
                LTOT = stat_pool.tile([128, NSLOT], F32, tag="LT")

                def out_proj(slot):
                    OUTS = out_pool.tile([128, D], F32, tag="outs")
                    for eh in range(2):
                        ps = mm_psum.tile([128, 512], F32, tag="mm")
                        for dc in range(EC):
                            nc.tensor.matmul(
                                ps[:, :],
                                CTXT[:, dc, slot * 128 : (slot + 1) * 128],
                                WO[:, dc, eh * 512 : (eh + 1) * 512],
                                start=(dc == 0),
                                stop=(dc == EC - 1),
                            )
                        nc.vector.scalar_tensor_tensor(
                            out=OUTS[:, eh * 512 : (eh + 1) * 512],
                            in0=ps[:, :],
                            scalar=RL[:, slot : slot + 1],
                            in1=BOF[:, eh * 512 : (eh + 1) * 512],
                            op0=mybir.AluOpType.mult,
                            op1=mybir.AluOpType.add,
                        )
                        # per-half store on the scalar queue (idle in phase B)
                        nc.scalar.dma_start(
                            out=out_d[
                                slot * 128 : (slot + 1) * 128,
                                eh * 512 : (eh + 1) * 512,
                            ],
                            in_=OUTS[:, eh * 512 : (eh + 1) * 512],
                        )

                for g in range(2):
                    ATT_T = attT_pool.tile([128, NB, 512], BF16, tag="attT")
                    # widest slot first: its long scores+softmax+transpose
                    # chain overlaps the narrower slots; the group's attnV
                    # gate becomes the narrowest slot's short tail.
                    for j in (3, 2, 1, 0):
                        slot = g * 4 + j
                        T = _slot_T(slot)
                        ATT = att_pool.tile([128, S], BF16, tag="att")

                        nparts = (T + 1023) // 1024
                        parts = []
                        for p in range(nparts):
                            w = min(1024, T - p * 1024)
                            sc = sc_psum.tile([128, 1024], F32, tag="sc")
                            parts.append((sc, w))
                        # ec-outer: one LDWEIGHTS per ec covers the whole row
                        for ec in range(EC):
                            for p, (sc, w) in enumerate(parts):
                                for c0 in range(0, w, 512):
                                    cw = min(512, w - c0)
                                    a0 = p * 1024 + c0
                                    nc.tensor.matmul(
                                        sc[:, c0 : c0 + cw],
                                        QT[:, ec, slot * 128 : (slot + 1) * 128],
                                        KT[:, ec, a0 : a0 + cw],
                                        start=(ec == 0),
                                        stop=(ec == EC - 1),
                                    )
                        lsc, lw = parts[-1]
                        nc.vector.tensor_tensor(
                            out=lsc[:, lw - 256 : lw],
                            in0=lsc[:, lw - 256 : lw],
                            in1=MASK[:, slot, :],
                            op=mybir.AluOpType.add,
                        )
                        # no max-subtraction: scores ~ N(0,1) (|s|<~6), so
                        # exp() is overflow-safe and each part's exp +
                        # transpose fires as soon as that part's scores land
                        for p, (sc, w) in enumerate(parts):
                            nc.scalar.activation(
                                ATT[:, p * 1024 : p * 1024 + w],
                                sc[:, :w],
                                mybir.ActivationFunctionType.Exp,
                                bias=0.0,
                                scale=1.0,
                                accum_out=LSUM[:, 2 * slot + p : 2 * slot + p + 1],
                            )
                            nc.sync.dma_start_transpose(
                                ATT_T[:, p * 8 : p * 8 + w // 128, j * 128 : (j + 1) * 128],
                                ATT[:, p * 1024 : p * 1024 + w],
                            )
                        if nparts == 2:
                            nc.vector.tensor_tensor(
                                out=LTOT[:, slot : slot + 1],
                                in0=LSUM[:, 2 * slot : 2 * slot + 1],
                                in1=LSUM[:, 2 * slot + 1 : 2 * slot + 2],
                                op=mybir.AluOpType.add,
                            )
                            nc.vector.reciprocal(
                                RL[:, slot : slot + 1], LTOT[:, slot : slot + 1]
                            )
                        else:
                            nc.vector.reciprocal(
                                RL[:, slot : slot + 1],
                                LSUM[:, 2 * slot : 2 * slot + 1],
                            )

                    ntg = _slot_T(g * 4 + 3) // 128
                    for dc in range(EC):
                        ps = mm_psum.tile([128, 512], F32, tag="mm")
                        for tcn in range(ntg):
                            jmin = 0
                            for jj in range(4):
                                if 256 * (g * 4 + jj + 1) >= 128 * (tcn + 1):
                                    jmin = jj
                                    break
                            scol = jmin * 128
                            nc.tensor.matmul(
                                ps[:, scol:512],
                                V[:, tcn, dc * 128 : (dc + 1) * 128],
                                ATT_T[:, tcn, scol:512],
                                start=(tcn == 0),
                                stop=(tcn == ntg - 1),
                            )
                        nc.vector.tensor_copy(
                            CTXT[:, dc, g * 512 : (g + 1) * 512], ps[:, :]
                        )
                    for j in range(4):
                        out_proj(g * 4 + j)

    nc.compile()
    return nc


def _core_blocks(core):
    parity = core % 2  # even core (pair rank 0) -> even blocks
    return [2 * s + parity for s in range(NSLOT)]


def _make_in_maps(x, Wq, bq, Wk, bk, Wv, bv, Wo, bo):
    bf = ml_dtypes.bfloat16

    def wt_layout(W):
        return np.ascontiguousarray(
            W.T.astype(bf).reshape(EC, 128, D).transpose(1, 0, 2)
        )

    def xT_layout(xrows, n):
        return np.ascontiguousarray(
            xrows.T.astype(bf).reshape(EC, 128, n).transpose(1, 0, 2)
        )

    wq_l, wk_l, wv_l, wo_l = (wt_layout(W) for W in (Wq, Wk, Wv, Wo))
    bq_l = np.ascontiguousarray(bq.reshape(EC, 128).T.astype(np.float32))
    bk_l = np.ascontiguousarray(bk.reshape(EC, 128).T.astype(np.float32))
    bv_l = np.ascontiguousarray(bv.reshape(1, D).astype(np.float32))
    bo_l = np.ascontiguousarray(bo.reshape(1, D).astype(np.float32))

    in_maps = []
    for core in range(8):
        b = core // 2
        blocks = _core_blocks(core)
        xb = np.asarray(x[b], dtype=np.float32)
        # parity-compact x: own blocks (used for Q and K-own)
        xp = np.concatenate([xb[bl * 128 : (bl + 1) * 128] for bl in blocks], axis=0)
        # E-compact | O-compact full x (used for V)
        xeo = np.concatenate(
            [xb[bl * 128 : (bl + 1) * 128] for bl in range(0, NB, 2)]
            + [xb[bl * 128 : (bl + 1) * 128] for bl in range(1, NB, 2)],
            axis=0,
        )
        # tail mask over the last 256 logical key columns of each slot
        mask = np.zeros((128, NSLOT, 256), np.float32)
        r = np.arange(128)[:, None]
        jj = np.arange(256)[None, :]
        for s_i, bl in enumerate(blocks):
            lim = bl * 128 + r
            t_idx = 256 * s_i + jj
            mask[:, s_i, :] = np.where(t_idx <= lim, 0.0, NEG)
        in_maps.append(
            {
                "xpT": xT_layout(xp, 1024),
                "xeoT": xT_layout(xeo, S),
                "wqT": wq_l,
                "wkT": wk_l,
                "wvT": wv_l,
                "woT": wo_l,
                "bq": bq_l,
                "bk": bk_l,
                "bv": bv_l,
                "bo": bo_l,
                "mask": mask,
            }
        )
    return in_maps


def _run(inputs, trace=False):
    global _compiled
    if _compiled is None:
        _compiled = _build()
    nc = _compiled
    in_maps = _make_in_maps(**inputs)
    res = run_bass_kernel_spmd(nc, in_maps, core_ids=list(range(8)), trace=trace)
    out = np.zeros((B, S, D), np.float32)
    for core in range(8):
        b = core // 2
        o = res.results[core]["out"]
        for s_i, bl in enumerate(_core_blocks(core)):
            out[b, bl * 128 : (bl + 1) * 128, :] = o[s_i * 128 : (s_i + 1) * 128, :]
    return out, res


def kernel(**inputs):
    out, _ = _run(inputs, trace=False)
    return out
